# revision 22
# baseline (speedup 1.0000x reference)
"""MoE feed-forward (top-2 routing + shared expert) on 8 Trainium2 cores.

Strategy (expert parallel):
  - Host computes the router (tiny [T,D]@[D,E] matmul), top-2 expert ids and
    renormalized gates, then dispatches each expert's tokens (transposed,
    capacity-padded) to the core that owns that expert's weights.
  - Core e computes  ye = (silu(xe@w1_e) * (xe@w3_e)) @ w2_e, row-scaled by the
    gate, plus a 1/8 token-slice of the always-active shared expert.
  - Host scatter-adds routed outputs into the shared-expert output.

All matmul operands are bf16 (fp32 PSUM accumulation), which runs the PE at
full rate AND halves HBM traffic vs f32. That lets every weight be fetched
from HBM exactly once: phase 1 iterates h-blocks in the OUTER loop (streaming
each w1/w3 tile once, all token chunks inner) while the per-token activations
and the silu(h1)*h3 gate tensor stay SBUF-resident for the whole block.
Total per-core HBM traffic is ~27MB (vs ~98MB for the f32r per-chunk
streaming variant), well under the ~266us of matmul work, so the kernel is
tensor-engine-bound.
"""

import numpy as np
import ml_dtypes

import concourse.bass as bass
import concourse.mybir as mybir
import concourse.tile as tile
from concourse import bacc
from concourse.bass_utils import run_bass_kernel_spmd

P = 128
N_CORES = 8
F32 = mybir.dt.float32
BF16 = mybir.dt.bfloat16
AF = mybir.ActivationFunctionType
NP_BF16 = ml_dtypes.bfloat16

# h-tiles of w1/w3 fetched per DMA (bigger transfers, fewer descriptors)
H_BLOCK = 2


def _chunk_widths(n):
    """Split n into the minimal number of chunks of width <=512 (PSUM bank),
    each >=~250 so the bf16 weight load (~97ns) hides under the column
    stream. The first chunk is kept small (256) when possible: it is on the
    DMA critical path at kernel start."""
    m = -(-n // 512)
    if m >= 2 and n - 256 <= 512 * (m - 1):
        rest, k = n - 256, m - 1
        return [256] + [rest // k + (1 if i < rest % k else 0) for i in range(k)]
    return [n // m + (1 if i < n % m else 0) for i in range(m)]


def _swiglu_block(
    tc,
    pools,
    xt,
    n_rows,
    w1_ap,
    w3_ap,
    w2_ap,
    out_ap,
    ge_tile,
    use_silu=True,
    post_w_loads=None,
):
    """Emit one SwiGLU y = (silu(x@w1) * (x@w3)) @ w2 over n_rows tokens.

    xt: SBUF tile [P, KD, n_rows] bf16 (DMA'd by caller).
    out_ap: [n_rows, D] f32 dram. If ge_tile ([P, n_rows//P] f32) is given,
    output rows are scaled by it (per-token gate).
    """
    nc = tc.nc
    D = out_ap.shape[1]
    KD = xt.shape[1]
    H = (w1_ap.shape[1] * P) // D
    KH = H // P
    ND = D // 512  # output free-dim tiles
    NHB = KH // H_BLOCK

    w2pool, wpool, gpool, spool, opool, pp1, pp3, ppo = pools

    hbsz = KD * H_BLOCK * P  # packed cols per h-block

    def _wsrc(ap, hb):
        return ap[:, hb * hbsz : (hb + 1) * hbsz].rearrange(
            "p (k m) -> p k m", k=KD
        )

    chunks = []
    off = 0
    for cw in _chunk_widths(n_rows):
        chunks.append((off, cw))
        off += cw

    # ---- phase 1: gT[h, c] = silu(h1T) * h3T, h-block outer so each
    # w1/w3 tile is fetched from HBM exactly once ----
    gt = gpool.tile([P, KH, n_rows], BF16, tag="gT", name="gt")
    w2t = w2pool.tile([P, KH, D], BF16, tag="w2res", name="w2t")
    for hb in range(NHB):
        w1t = wpool.tile([P, KD, H_BLOCK * P], BF16, tag="w1t", name="w1t")
        nc.sync.dma_start(w1t[:], _wsrc(w1_ap, hb))
        if hb == 0 and post_w_loads is not None:
            post_w_loads(chunks, 0)  # first x chunk: first-matmul critical path
        w3t = wpool.tile([P, KD, H_BLOCK * P], BF16, tag="w3t", name="w3t")
        nc.sync.dma_start(w3t[:], _wsrc(w3_ap, hb))
        if hb == 0 and post_w_loads is not None:
            post_w_loads(chunks, 1)  # remaining activations
        # prefetch w2 halves during the last two h-blocks so phase 2
        # starts without a DMA bubble
        for dn in range(ND):
            if hb == NHB - ND + dn:
                nc.sync.dma_start(
                    w2t[:, :, dn * 512 : (dn + 1) * 512],
                    w2_ap[:, dn * KH * 512 : (dn + 1) * KH * 512].rearrange(
                        "p (k m) -> p k m", k=KH
                    ),
                )
        for c0, cw in chunks:
            for hi in range(H_BLOCK):
                h = hb * H_BLOCK + hi
                p1 = pp1.tile([P, 512], F32, tag="p1", name="p1")[:, :cw]
                p3 = pp3.tile([P, 512], F32, tag="p3", name="p3")[:, :cw]
                for k in range(KD):
                    nc.tensor.matmul(
                        p1,
                        w1t[:, k, hi * P : (hi + 1) * P],
                        xt[:, k, c0 : c0 + cw],
                        start=(k == 0),
                        stop=(k == KD - 1),
                    )
                for k in range(KD):
                    nc.tensor.matmul(
                        p3,
                        w3t[:, k, hi * P : (hi + 1) * P],
                        xt[:, k, c0 : c0 + cw],
                        start=(k == 0),
                        stop=(k == KD - 1),
                    )
                if use_silu:
                    s1 = spool.tile([P, 512], BF16, tag="s1", name="s1")[:, :cw]
                    nc.scalar.activation(s1, p1, AF.Silu)
                    nc.vector.tensor_mul(gt[:, h, c0 : c0 + cw], s1, p3)
                else:  # silu(a) = a * sigmoid(a); CoreSim has no Silu table
                    s1 = spool.tile([P, 512], F32, tag="s1f", name="s1f")[:, :cw]
                    s2 = spool.tile([P, 512], F32, tag="s2f", name="s2f")[:, :cw]
                    nc.scalar.activation(s1, p1, AF.Sigmoid)
                    nc.vector.tensor_mul(s2, p1, p3)
                    nc.vector.tensor_mul(gt[:, h, c0 : c0 + cw], s2, s1)

    # ---- phase 2: out = gT.T @ w2, w2 SBUF-resident (prefetched above) ----
    for dn in range(ND):
        for ct in range(-(-n_rows // P)):
            rows = min(P, n_rows - ct * P)
            po = ppo.tile([P, 512], F32, tag="po", name="po")[:rows]
            for kh in range(KH):
                nc.tensor.matmul(
                    po,
                    gt[:, kh, ct * P : ct * P + rows],
                    w2t[:, kh, dn * 512 : (dn + 1) * 512],
                    start=(kh == 0),
                    stop=(kh == KH - 1),
                )
            ot = opool.tile([P, 512], F32, tag="ot", name="ot")[:rows]
            if ge_tile is not None:
                nc.vector.tensor_scalar_mul(
                    ot, po, ge_tile[:rows, ct : ct + 1]
                )
            else:
                nc.vector.tensor_copy(ot, po)
            nc.sync.dma_start(
                out_ap[ct * P : ct * P + rows, dn * 512 : (dn + 1) * 512],
                ot,
            )


def build_moe_program(D, H, C, S, use_silu=True):
    """SPMD program: routed expert over C capacity rows + shared expert over
    S token-slice rows. Same NEFF on all 8 cores, per-core input data."""
    nc = bacc.Bacc(
        "TRN2", target_bir_lowering=False, debug=False, num_devices=N_CORES
    )
    KD = D // P

    def din(name, shape, dt=BF16):
        return nc.dram_tensor(name, shape, dt, kind="ExternalInput").ap()

    def dout(name, shape):
        return nc.dram_tensor(name, shape, F32, kind="ExternalOutput").ap()

    CP = -(-C // P)
    xeT = din("xeT", [P, KD * C])
    ge = din("ge", [P, CP], F32)
    xsT = din("xsT", [P, KD * S])
    w1 = din("w1", [P, KD * H])
    w3 = din("w3", [P, KD * H])
    w2 = din("w2", [P, H * D // P])
    sw1 = din("sw1", [P, KD * H])
    sw3 = din("sw3", [P, KD * H])
    sw2 = din("sw2", [P, H * D // P])
    ye = dout("ye", [C, D])
    se = dout("se", [S, D])

    with tile.TileContext(nc) as tc:
        from contextlib import ExitStack

        with ExitStack() as ctx:
            xepool = ctx.enter_context(tc.tile_pool(name="xeT", bufs=1))
            xspool = ctx.enter_context(tc.tile_pool(name="xsT", bufs=1))
            gepool = ctx.enter_context(tc.tile_pool(name="gate", bufs=1))
            # gT and w2res are bufs=1: the shared block reuses the routed
            # block's buffer (its writes serialize after the routed phase-2
            # reads, which matches the PE's program order anyway)
            pools = (
                ctx.enter_context(tc.tile_pool(name="w2res", bufs=1)),
                ctx.enter_context(tc.tile_pool(name="wstream", bufs=4)),
                ctx.enter_context(tc.tile_pool(name="gT", bufs=1)),
                ctx.enter_context(tc.tile_pool(name="stemp", bufs=2)),
                ctx.enter_context(tc.tile_pool(name="otile", bufs=3)),
                ctx.enter_context(tc.tile_pool(name="ps1", bufs=2, space="PSUM")),
                ctx.enter_context(tc.tile_pool(name="ps3", bufs=2, space="PSUM")),
                ctx.enter_context(tc.tile_pool(name="pso", bufs=2, space="PSUM")),
            )
            # activation tiles for both blocks stay SBUF-resident; their DMAs
            # are emitted right after the first w1/w3 tile DMAs (see
            # post_w_loads) so the DMA ring serves the PE's critical path
            # first: w-hb0, then x chunk-by-chunk (narrowest chunk first)
            xet = xepool.tile([P, KD, C], BF16, tag="xeT", name="xet")
            xesrc = xeT.rearrange("p (k c) -> p k c", k=KD)
            xst = xspool.tile([P, KD, S], BF16, tag="xsT", name="xst")
            xssrc = xsT.rearrange("p (k c) -> p k c", k=KD)
            get = gepool.tile([P, CP], F32, tag="ge", name="get")

            def load_acts(chunks, stage):
                if stage == 0:
                    c0, cw = chunks[0]
                    for k0 in range(0, KD, 2):
                        nc.sync.dma_start(
                            xet[:, k0 : k0 + 2, c0 : c0 + cw],
                            xesrc[:, k0 : k0 + 2, c0 : c0 + cw],
                        )
                    return
                for c0, cw in chunks[1:]:
                    for k0 in range(0, KD, 2):
                        nc.sync.dma_start(
                            xet[:, k0 : k0 + 2, c0 : c0 + cw],
                            xesrc[:, k0 : k0 + 2, c0 : c0 + cw],
                        )
                for k0 in range(0, KD, 4):
                    nc.sync.dma_start(
                        xst[:, k0 : k0 + 4, :], xssrc[:, k0 : k0 + 4, :]
                    )
                nc.sync.dma_start(get[:], ge)

            _swiglu_block(
                tc, pools, xet, C, w1, w3, w2, ye, get, use_silu, load_acts
            )
            _swiglu_block(tc, pools, xst, S, sw1, sw3, sw2, se, None, use_silu)

    nc.compile()
    return nc


_PROGRAM_CACHE = {}
LAST_RESULTS = None  # BassKernelResults of the most recent device run (for test.py)


def _get_program(D, H, C, S):
    key = (D, H, C, S)
    if key not in _PROGRAM_CACHE:
        _PROGRAM_CACHE[key] = build_moe_program(D, H, C, S)
    return _PROGRAM_CACHE[key]


def _pack_xT(xmat):
    """[n, D] row-major tokens -> [P, KD*n] partition-major k-major layout:
    element [p, k*n + c] = xmat[c, k*P + p]."""
    n, Dx = xmat.shape
    KD = Dx // P
    return np.ascontiguousarray(
        xmat.reshape(n, KD, P).transpose(2, 1, 0).reshape(P, KD * n)
    ).astype(NP_BF16)


def _pack_w13(w):
    """[D, H] -> [P, (D//P)*H] h-block-major bf16: each h-block's weights are
    one contiguous run per partition."""
    Dw, Hw = w.shape
    KD = Dw // P
    nhb = Hw // (H_BLOCK * P)
    return np.ascontiguousarray(
        w.reshape(KD, P, nhb, H_BLOCK * P)
        .transpose(1, 2, 0, 3)
        .reshape(P, KD * Hw)
    ).astype(NP_BF16)


def _pack_w2(w):
    """[H, D] -> [P, H*D//P] dn-major bf16: each 512-wide D-column half is one
    contiguous run per partition."""
    Hw, Dw = w.shape
    KH = Hw // P
    ND = Dw // 512
    return np.ascontiguousarray(
        w.reshape(KH, P, ND, 512).transpose(1, 2, 0, 3).reshape(P, Hw * Dw // P)
    ).astype(NP_BF16)


def _route(xf, w_router):
    """Top-2 routing identical (up to fp rounding) to the jax reference."""
    logits = xf @ w_router.astype(np.float32)  # [T, E]
    # softmax is monotone: top-2 of probs == top-2 of logits, stable ties
    top2 = np.argsort(-logits, axis=1, kind="stable")[:, :2]  # [T, 2]
    lv = np.take_along_axis(logits, top2, axis=1)
    ev = np.exp(lv - lv[:, 0:1])
    gates = ev / ev.sum(axis=1, keepdims=True)  # [T, 2] renormalized
    return top2, gates


def kernel(x, w_router, w1, w3, w2, sw1, sw3, sw2):
    B, SEQ, D = x.shape
    T = B * SEQ
    E, _, H = w1.shape
    assert E == N_CORES
    S = T // N_CORES

    x = np.asarray(x, dtype=np.float32)
    xf = np.ascontiguousarray(x.reshape(T, D))
    top2, gates = _route(xf, np.asarray(w_router, np.float32))

    # per-expert token lists + gate values
    flat_e = top2.ravel()  # slot 2t, 2t+1 -> token t
    flat_g = gates.ravel().astype(np.float32)
    order = np.argsort(flat_e, kind="stable")
    sorted_e = flat_e[order]
    starts = np.searchsorted(sorted_e, np.arange(E + 1))
    tok_by_e = [order[starts[e] : starts[e + 1]] >> 1 for e in range(E)]
    gate_by_e = [flat_g[order[starts[e] : starts[e + 1]]] for e in range(E)]
    counts = np.diff(starts)

    # capacity: exact max expert load (every core pays C rows of compute, so
    # don't round up); small floor keeps degenerate routings compilable
    C = max(256, int(counts.max()))

    nc = _get_program(D, H, C, S)

    w1 = np.asarray(w1, np.float32)
    w3 = np.asarray(w3, np.float32)
    w2 = np.asarray(w2, np.float32)
    sw1p = _pack_w13(np.asarray(sw1, np.float32))
    sw3p = _pack_w13(np.asarray(sw3, np.float32))
    sw2p = _pack_w2(np.asarray(sw2, np.float32))

    in_maps = []
    for e in range(E):
        n_e = int(counts[e])
        xe_pad = np.zeros((C, D), np.float32)
        xe_pad[:n_e] = xf[tok_by_e[e]]
        xeT = _pack_xT(xe_pad)
        CP = -(-C // P)
        ge = np.zeros((CP * P,), np.float32)
        ge[:n_e] = gate_by_e[e]
        gep = np.ascontiguousarray(ge.reshape(CP, P).T)
        xsT = _pack_xT(xf[e * S : (e + 1) * S])
        in_maps.append(
            {
                "xeT": xeT,
                "ge": gep,
                "xsT": xsT,
                "w1": _pack_w13(w1[e]),
                "w3": _pack_w13(w3[e]),
                "w2": _pack_w2(w2[e]),
                "sw1": sw1p,
                "sw3": sw3p,
                "sw2": sw2p,
            }
        )

    global LAST_RESULTS
    LAST_RESULTS = run_bass_kernel_spmd(nc, in_maps, core_ids=list(range(N_CORES)))
    res = LAST_RESULTS.results

    out = np.empty((T, D), np.float32)
    for c in range(N_CORES):
        out[c * S : (c + 1) * S] = res[c]["se"]
    for e in range(E):
        n_e = int(counts[e])
        if n_e:
            out[tok_by_e[e]] += res[e]["ye"][:n_e]
    return out.reshape(B, SEQ, D)


# revision 33
# speedup vs baseline: 1.0169x; 1.0169x over previous
"""MoE feed-forward (top-2 routing + shared expert) on 8 Trainium2 cores.

Strategy (expert parallel):
  - Host computes the router (tiny [T,D]@[D,E] matmul), top-2 expert ids and
    renormalized gates, then dispatches each expert's tokens (transposed,
    capacity-padded) to the core that owns that expert's weights.
  - Core e computes  ye = (silu(xe@w1_e) * (xe@w3_e)) @ w2_e, row-scaled by the
    gate, plus a 1/8 token-slice of the always-active shared expert.
  - Host scatter-adds routed outputs into the shared-expert output.

All matmul operands are bf16 (fp32 PSUM accumulation), which runs the PE at
full rate AND halves HBM traffic vs f32. That lets every weight be fetched
from HBM exactly once: phase 1 iterates h-blocks in the OUTER loop (streaming
each w1/w3 tile once, all token chunks inner) while the per-token activations
and the silu(h1)*h3 gate tensor stay SBUF-resident for the whole block.
Per-core HBM traffic is ~27MB (vs ~98MB for an f32 per-chunk streaming
variant), well under the matmul time, so the kernel is tensor-engine-bound:
PE issue rate is width/2.4GHz per matmul with the ~97ns bf16 weight load
hidden (all chunk widths kept >=256), zero PE gaps >200ns after the ~12us
DMA ramp-in. Capacity is the EXACT max expert load (no 128-rounding): every
core pays C rows, so phase 1 scales with C and phase 2 with ceil(C/128).

Accuracy: bf16 quantization of weights+activations lands at ~4e-3
absmax-relative (budget 2e-2). fp8 e4m3 was measured at 3.6-6.9e-2 for
every subset of the matmuls and is excluded.
"""

import numpy as np
import ml_dtypes

import concourse.bass as bass
import concourse.mybir as mybir
import concourse.tile as tile
from concourse import bacc
from concourse.bass_utils import run_bass_kernel_spmd

P = 128
N_CORES = 8
F32 = mybir.dt.float32
BF16 = mybir.dt.bfloat16
AF = mybir.ActivationFunctionType
NP_BF16 = ml_dtypes.bfloat16

# h-tiles of w1/w3 fetched per DMA (bigger transfers, fewer descriptors)
H_BLOCK = 2


def _chunk_widths(n, small_first=True):
    """Split n into the minimal number of chunks of width <=512 (PSUM bank),
    each >=~250 so the bf16 weight load (~97ns) hides under the column
    stream. With small_first, the first chunk is kept at 256 when possible:
    it is on the DMA critical path at kernel start."""
    m = -(-n // 512)
    if small_first and m >= 2 and n - 256 <= 512 * (m - 1):
        rest, k = n - 256, m - 1
        return [256] + [rest // k + (1 if i < rest % k else 0) for i in range(k)]
    return [n // m + (1 if i < n % m else 0) for i in range(m)]


def _swiglu_block(
    tc,
    pools,
    xt,
    n_rows,
    w1_ap,
    w3_ap,
    w2_ap,
    out_ap,
    use_silu=True,
    post_w_loads=None,
):
    """Emit one SwiGLU yT = ((silu(x@w1) * (x@w3)) @ w2).T over n_rows tokens.

    xt: SBUF tile [P, KD, n_rows] bf16 (DMA'd by caller).
    out_ap: [D, n_rows] f32 dram, d-major (host transposes; tokens live on
    the matmul free axis in both phases so compute scales exactly with
    n_rows, with no padding to a 128-row PSUM tile). Per-token gates are
    applied by the host.
    """
    nc = tc.nc
    D = out_ap.shape[0]
    KD = xt.shape[1]
    H = (w1_ap.shape[1] * P) // D
    KH = H // P
    ND = D // 512  # output free-dim tiles
    NHB = KH // H_BLOCK

    w2pool, wpool, gpool, spool, opool, pp1, pp3, ppo = pools

    hbsz = KD * H_BLOCK * P  # packed cols per h-block

    def _wsrc(ap, hb):
        return ap[:, hb * hbsz : (hb + 1) * hbsz].rearrange(
            "p (k m) -> p k m", k=KD
        )

    chunks = []
    off = 0
    for cw in _chunk_widths(n_rows):
        chunks.append((off, cw))
        off += cw

    # ---- phase 1: gT[h, c] = silu(h1T) * h3T, h-block outer so each
    # w1/w3 tile is fetched from HBM exactly once ----
    gt = gpool.tile([P, KH, n_rows], BF16, tag="gT", name="gt")
    w2t = w2pool.tile([P, KH, D], BF16, tag="w2res", name="w2t")
    for hb in range(NHB):
        w1t = wpool.tile([P, KD, H_BLOCK * P], BF16, tag="w1t", name="w1t")
        nc.sync.dma_start(w1t[:], _wsrc(w1_ap, hb))
        if hb == 0 and post_w_loads is not None:
            post_w_loads(chunks, 0)  # first x chunk: first-matmul critical path
        w3t = wpool.tile([P, KD, H_BLOCK * P], BF16, tag="w3t", name="w3t")
        nc.sync.dma_start(w3t[:], _wsrc(w3_ap, hb))
        if hb == 0 and post_w_loads is not None:
            post_w_loads(chunks, 1)  # remaining activations
        # prefetch w2 halves during the last two h-blocks so phase 2
        # starts without a DMA bubble
        for dn in range(ND):
            if hb == NHB - ND + dn:
                nc.sync.dma_start(
                    w2t[:, :, dn * 512 : (dn + 1) * 512],
                    w2_ap[:, dn * KH * 512 : (dn + 1) * KH * 512].rearrange(
                        "p (k m) -> p k m", k=KH
                    ),
                )
        for c0, cw in chunks:
            for hi in range(H_BLOCK):
                h = hb * H_BLOCK + hi
                p1 = pp1.tile([P, 512], F32, tag="p1", name="p1")[:, :cw]
                p3 = pp3.tile([P, 512], F32, tag="p3", name="p3")[:, :cw]
                for k in range(KD):
                    nc.tensor.matmul(
                        p1,
                        w1t[:, k, hi * P : (hi + 1) * P],
                        xt[:, k, c0 : c0 + cw],
                        start=(k == 0),
                        stop=(k == KD - 1),
                    )
                for k in range(KD):
                    nc.tensor.matmul(
                        p3,
                        w3t[:, k, hi * P : (hi + 1) * P],
                        xt[:, k, c0 : c0 + cw],
                        start=(k == 0),
                        stop=(k == KD - 1),
                    )
                if use_silu:
                    s1 = spool.tile([P, 512], BF16, tag="s1", name="s1")[:, :cw]
                    nc.scalar.activation(s1, p1, AF.Silu)
                    nc.vector.tensor_mul(gt[:, h, c0 : c0 + cw], s1, p3)
                else:  # silu(a) = a * sigmoid(a); CoreSim has no Silu table
                    s1 = spool.tile([P, 512], F32, tag="s1f", name="s1f")[:, :cw]
                    s2 = spool.tile([P, 512], F32, tag="s2f", name="s2f")[:, :cw]
                    nc.scalar.activation(s1, p1, AF.Sigmoid)
                    nc.vector.tensor_mul(s2, p1, p3)
                    nc.vector.tensor_mul(gt[:, h, c0 : c0 + cw], s2, s1)

    # ---- phase 2: outT = w2.T @ gT, w2 SBUF-resident (prefetched above).
    # Stationary is a [h,128] w2 d-tile, moving is gt with tokens on the
    # free axis, so the stream is exactly n_rows columns per (d-tile, kh)
    # and the output lands d-major [D, n] (host transposes) ----
    chunks2 = []
    off = 0
    for cw in _chunk_widths(n_rows, small_first=False):
        chunks2.append((off, cw))
        off += cw
    for dt in range(D // P):
        for c0, cw in chunks2:
            po = ppo.tile([P, 512], F32, tag="po", name="po")[:, :cw]
            for kh in range(KH):
                nc.tensor.matmul(
                    po,
                    w2t[:, kh, dt * P : (dt + 1) * P],
                    gt[:, kh, c0 : c0 + cw],
                    start=(kh == 0),
                    stop=(kh == KH - 1),
                )
            ot = opool.tile([P, 512], F32, tag="ot", name="ot")[:, :cw]
            nc.vector.tensor_copy(ot, po)
            nc.sync.dma_start(
                out_ap[dt * P : (dt + 1) * P, c0 : c0 + cw],
                ot,
            )


def build_moe_program(D, H, C, S, use_silu=True):
    """SPMD program: routed expert over C capacity rows + shared expert over
    S token-slice rows. Same NEFF on all 8 cores, per-core input data."""
    nc = bacc.Bacc(
        "TRN2", target_bir_lowering=False, debug=False, num_devices=N_CORES
    )
    KD = D // P

    def din(name, shape, dt=BF16):
        return nc.dram_tensor(name, shape, dt, kind="ExternalInput").ap()

    def dout(name, shape):
        return nc.dram_tensor(name, shape, F32, kind="ExternalOutput").ap()

    xeT = din("xeT", [P, KD * C])
    xsT = din("xsT", [P, KD * S])
    w1 = din("w1", [P, KD * H])
    w3 = din("w3", [P, KD * H])
    w2 = din("w2", [P, H * D // P])
    sw1 = din("sw1", [P, KD * H])
    sw3 = din("sw3", [P, KD * H])
    sw2 = din("sw2", [P, H * D // P])
    ye = dout("ye", [D, C])
    se = dout("se", [D, S])

    with tile.TileContext(nc) as tc:
        from contextlib import ExitStack

        with ExitStack() as ctx:
            xepool = ctx.enter_context(tc.tile_pool(name="xeT", bufs=1))
            xspool = ctx.enter_context(tc.tile_pool(name="xsT", bufs=1))
            # gT and w2res are bufs=1: the shared block reuses the routed
            # block's buffer (its writes serialize after the routed phase-2
            # reads, which matches the PE's program order anyway)
            pools = (
                ctx.enter_context(tc.tile_pool(name="w2res", bufs=1)),
                ctx.enter_context(tc.tile_pool(name="wstream", bufs=4)),
                ctx.enter_context(tc.tile_pool(name="gT", bufs=1)),
                ctx.enter_context(tc.tile_pool(name="stemp", bufs=2)),
                ctx.enter_context(tc.tile_pool(name="otile", bufs=3)),
                ctx.enter_context(tc.tile_pool(name="ps1", bufs=2, space="PSUM")),
                ctx.enter_context(tc.tile_pool(name="ps3", bufs=2, space="PSUM")),
                ctx.enter_context(tc.tile_pool(name="pso", bufs=2, space="PSUM")),
            )
            # activation tiles for both blocks stay SBUF-resident; their DMAs
            # are emitted right after the first w1/w3 tile DMAs (see
            # post_w_loads) so the DMA ring serves the PE's critical path
            # first: w-hb0, then x chunk-by-chunk (narrowest chunk first)
            xet = xepool.tile([P, KD, C], BF16, tag="xeT", name="xet")
            xesrc = xeT.rearrange("p (k c) -> p k c", k=KD)
            xst = xspool.tile([P, KD, S], BF16, tag="xsT", name="xst")
            xssrc = xsT.rearrange("p (k c) -> p k c", k=KD)

            def load_acts(chunks, stage):
                if stage == 0:
                    c0, cw = chunks[0]
                    for k0 in range(0, KD, 2):
                        nc.sync.dma_start(
                            xet[:, k0 : k0 + 2, c0 : c0 + cw],
                            xesrc[:, k0 : k0 + 2, c0 : c0 + cw],
                        )
                    return
                for c0, cw in chunks[1:]:
                    for k0 in range(0, KD, 2):
                        nc.sync.dma_start(
                            xet[:, k0 : k0 + 2, c0 : c0 + cw],
                            xesrc[:, k0 : k0 + 2, c0 : c0 + cw],
                        )
                for k0 in range(0, KD, 4):
                    nc.sync.dma_start(
                        xst[:, k0 : k0 + 4, :], xssrc[:, k0 : k0 + 4, :]
                    )

            _swiglu_block(
                tc, pools, xet, C, w1, w3, w2, ye, use_silu, load_acts
            )
            _swiglu_block(tc, pools, xst, S, sw1, sw3, sw2, se, use_silu)

    nc.compile()
    return nc


_PROGRAM_CACHE = {}
LAST_RESULTS = None  # BassKernelResults of the most recent device run (for test.py)


def _get_program(D, H, C, S):
    key = (D, H, C, S)
    if key not in _PROGRAM_CACHE:
        _PROGRAM_CACHE[key] = build_moe_program(D, H, C, S)
    return _PROGRAM_CACHE[key]


def _pack_xT(xmat):
    """[n, D] row-major tokens -> [P, KD*n] partition-major k-major layout:
    element [p, k*n + c] = xmat[c, k*P + p]."""
    n, Dx = xmat.shape
    KD = Dx // P
    return np.ascontiguousarray(
        xmat.reshape(n, KD, P).transpose(2, 1, 0).reshape(P, KD * n)
    ).astype(NP_BF16)


def _pack_w13(w):
    """[D, H] -> [P, (D//P)*H] h-block-major bf16: each h-block's weights are
    one contiguous run per partition."""
    Dw, Hw = w.shape
    KD = Dw // P
    nhb = Hw // (H_BLOCK * P)
    return np.ascontiguousarray(
        w.reshape(KD, P, nhb, H_BLOCK * P)
        .transpose(1, 2, 0, 3)
        .reshape(P, KD * Hw)
    ).astype(NP_BF16)


def _pack_w2(w):
    """[H, D] -> [P, H*D//P] dn-major bf16: each 512-wide D-column half is one
    contiguous run per partition."""
    Hw, Dw = w.shape
    KH = Hw // P
    ND = Dw // 512
    return np.ascontiguousarray(
        w.reshape(KH, P, ND, 512).transpose(1, 2, 0, 3).reshape(P, Hw * Dw // P)
    ).astype(NP_BF16)


def _route(xf, w_router):
    """Top-2 routing identical (up to fp rounding) to the jax reference."""
    logits = xf @ w_router.astype(np.float32)  # [T, E]
    # softmax is monotone: top-2 of probs == top-2 of logits, stable ties
    top2 = np.argsort(-logits, axis=1, kind="stable")[:, :2]  # [T, 2]
    lv = np.take_along_axis(logits, top2, axis=1)
    ev = np.exp(lv - lv[:, 0:1])
    gates = ev / ev.sum(axis=1, keepdims=True)  # [T, 2] renormalized
    return top2, gates


def kernel(x, w_router, w1, w3, w2, sw1, sw3, sw2):
    B, SEQ, D = x.shape
    T = B * SEQ
    E, _, H = w1.shape
    assert E == N_CORES
    S = T // N_CORES

    x = np.asarray(x, dtype=np.float32)
    xf = np.ascontiguousarray(x.reshape(T, D))
    top2, gates = _route(xf, np.asarray(w_router, np.float32))

    # per-expert token lists + gate values
    flat_e = top2.ravel()  # slot 2t, 2t+1 -> token t
    flat_g = gates.ravel().astype(np.float32)
    order = np.argsort(flat_e, kind="stable")
    sorted_e = flat_e[order]
    starts = np.searchsorted(sorted_e, np.arange(E + 1))
    tok_by_e = [order[starts[e] : starts[e + 1]] >> 1 for e in range(E)]
    gate_by_e = [flat_g[order[starts[e] : starts[e + 1]]] for e in range(E)]
    counts = np.diff(starts)

    # capacity: exact max expert load (every core pays C rows of compute, so
    # don't round up); small floor keeps degenerate routings compilable
    C = max(256, int(counts.max()))

    nc = _get_program(D, H, C, S)

    w1 = np.asarray(w1, np.float32)
    w3 = np.asarray(w3, np.float32)
    w2 = np.asarray(w2, np.float32)
    sw1p = _pack_w13(np.asarray(sw1, np.float32))
    sw3p = _pack_w13(np.asarray(sw3, np.float32))
    sw2p = _pack_w2(np.asarray(sw2, np.float32))

    in_maps = []
    for e in range(E):
        n_e = int(counts[e])
        xe_pad = np.zeros((C, D), np.float32)
        xe_pad[:n_e] = xf[tok_by_e[e]]
        xeT = _pack_xT(xe_pad)
        xsT = _pack_xT(xf[e * S : (e + 1) * S])
        in_maps.append(
            {
                "xeT": xeT,
                "xsT": xsT,
                "w1": _pack_w13(w1[e]),
                "w3": _pack_w13(w3[e]),
                "w2": _pack_w2(w2[e]),
                "sw1": sw1p,
                "sw3": sw3p,
                "sw2": sw2p,
            }
        )

    global LAST_RESULTS
    LAST_RESULTS = run_bass_kernel_spmd(nc, in_maps, core_ids=list(range(N_CORES)))
    res = LAST_RESULTS.results

    # device outputs are d-major [D, n]; gates are applied here (they can't
    # broadcast along the matmul free axis on-device)
    out = np.empty((T, D), np.float32)
    for c in range(N_CORES):
        out[c * S : (c + 1) * S] = res[c]["se"].T
    for e in range(E):
        n_e = int(counts[e])
        if n_e:
            out[tok_by_e[e]] += res[e]["ye"][:, :n_e].T * gate_by_e[e][:, None]
    return out.reshape(B, SEQ, D)


# revision 34
# speedup vs baseline: 1.0215x; 1.0046x over previous
"""MoE feed-forward (top-2 routing + shared expert) on 8 Trainium2 cores.

Strategy (expert parallel):
  - Host computes the router (tiny [T,D]@[D,E] matmul), top-2 expert ids and
    renormalized gates, then dispatches each expert's tokens (transposed,
    capacity-padded) to the core that owns that expert's weights.
  - Core e computes  ye = (silu(xe@w1_e) * (xe@w3_e)) @ w2_e, row-scaled by the
    gate, plus a 1/8 token-slice of the always-active shared expert.
  - Host scatter-adds routed outputs into the shared-expert output.

All matmul operands are bf16 (fp32 PSUM accumulation), which runs the PE at
full rate AND halves HBM traffic vs f32. That lets every weight be fetched
from HBM exactly once: phase 1 iterates h-blocks in the OUTER loop (streaming
each w1/w3 tile once, all token chunks inner) while the per-token activations
and the silu(h1)*h3 gate tensor stay SBUF-resident for the whole block.
Per-core HBM traffic is ~27MB (vs ~98MB for an f32 per-chunk streaming
variant), well under the matmul time, so the kernel is tensor-engine-bound:
PE issue rate is width/2.4GHz per matmul with the ~97ns bf16 weight load
hidden (all chunk widths kept >=256), zero PE gaps >200ns after the ~12us
DMA ramp-in. Capacity is the EXACT max expert load (no 128-rounding), and
tokens ride the matmul FREE axis in BOTH phases (phase 2 keeps w2 d-tiles
stationary and streams gt, emitting d-major [D, n] outputs that the host
transposes and gate-scales), so compute scales exactly with token count --
no padding to 128-row PSUM tiles anywhere.

Accuracy: bf16 quantization of weights+activations lands at ~4e-3
absmax-relative (budget 2e-2). fp8 e4m3 was measured at 3.6-6.9e-2 for
every subset of the matmuls and is excluded.
"""

import numpy as np
import ml_dtypes

import concourse.bass as bass
import concourse.mybir as mybir
import concourse.tile as tile
from concourse import bacc
from concourse.bass_utils import run_bass_kernel_spmd

P = 128
N_CORES = 8
F32 = mybir.dt.float32
BF16 = mybir.dt.bfloat16
AF = mybir.ActivationFunctionType
NP_BF16 = ml_dtypes.bfloat16

# h-tiles of w1/w3 fetched per DMA (bigger transfers, fewer descriptors)
H_BLOCK = 2


def _chunk_widths(n, small_first=True):
    """Split n into the minimal number of chunks of width <=512 (PSUM bank),
    each >=~250 so the bf16 weight load (~97ns) hides under the column
    stream. With small_first, the first chunk is kept at 256 when possible:
    it is on the DMA critical path at kernel start."""
    m = -(-n // 512)
    if small_first and m >= 2 and n - 256 <= 512 * (m - 1):
        rest, k = n - 256, m - 1
        return [256] + [rest // k + (1 if i < rest % k else 0) for i in range(k)]
    return [n // m + (1 if i < n % m else 0) for i in range(m)]


def _swiglu_block(
    tc,
    pools,
    xt,
    n_rows,
    w1_ap,
    w3_ap,
    w2_ap,
    out_ap,
    use_silu=True,
    post_w_loads=None,
):
    """Emit one SwiGLU yT = ((silu(x@w1) * (x@w3)) @ w2).T over n_rows tokens.

    xt: SBUF tile [P, KD, n_rows] bf16 (DMA'd by caller).
    out_ap: [D, n_rows] f32 dram, d-major (host transposes; tokens live on
    the matmul free axis in both phases so compute scales exactly with
    n_rows, with no padding to a 128-row PSUM tile). Per-token gates are
    applied by the host.
    """
    nc = tc.nc
    D = out_ap.shape[0]
    KD = xt.shape[1]
    H = (w1_ap.shape[1] * P) // D
    KH = H // P
    ND = D // 512  # output free-dim tiles
    NHB = KH // H_BLOCK

    w2pool, wpool, gpool, spool, opool, pp1, pp3, ppo = pools

    hbsz = KD * H_BLOCK * P  # packed cols per h-block

    def _wsrc(ap, hb):
        return ap[:, hb * hbsz : (hb + 1) * hbsz].rearrange(
            "p (k m) -> p k m", k=KD
        )

    chunks = []
    off = 0
    for cw in _chunk_widths(n_rows):
        chunks.append((off, cw))
        off += cw

    # ---- phase 1: gT[h, c] = silu(h1T) * h3T, h-block outer so each
    # w1/w3 tile is fetched from HBM exactly once ----
    gt = gpool.tile([P, KH, n_rows], BF16, tag="gT", name="gt")
    w2t = w2pool.tile([P, KH, D], BF16, tag="w2res", name="w2t")
    for hb in range(NHB):
        w1t = wpool.tile([P, KD, H_BLOCK * P], BF16, tag="w1t", name="w1t")
        nc.sync.dma_start(w1t[:], _wsrc(w1_ap, hb))
        if hb == 0 and post_w_loads is not None:
            post_w_loads(chunks, 0)  # first x chunk: first-matmul critical path
        w3t = wpool.tile([P, KD, H_BLOCK * P], BF16, tag="w3t", name="w3t")
        nc.sync.dma_start(w3t[:], _wsrc(w3_ap, hb))
        if hb == 0 and post_w_loads is not None:
            post_w_loads(chunks, 1)  # remaining activations
        # prefetch w2 halves during the last two h-blocks so phase 2
        # starts without a DMA bubble
        for dn in range(ND):
            if hb == NHB - ND + dn:
                nc.sync.dma_start(
                    w2t[:, :, dn * 512 : (dn + 1) * 512],
                    w2_ap[:, dn * KH * 512 : (dn + 1) * KH * 512].rearrange(
                        "p (k m) -> p k m", k=KH
                    ),
                )
        for c0, cw in chunks:
            for hi in range(H_BLOCK):
                h = hb * H_BLOCK + hi
                p1 = pp1.tile([P, 512], F32, tag="p1", name="p1")[:, :cw]
                p3 = pp3.tile([P, 512], F32, tag="p3", name="p3")[:, :cw]
                for k in range(KD):
                    nc.tensor.matmul(
                        p1,
                        w1t[:, k, hi * P : (hi + 1) * P],
                        xt[:, k, c0 : c0 + cw],
                        start=(k == 0),
                        stop=(k == KD - 1),
                    )
                for k in range(KD):
                    nc.tensor.matmul(
                        p3,
                        w3t[:, k, hi * P : (hi + 1) * P],
                        xt[:, k, c0 : c0 + cw],
                        start=(k == 0),
                        stop=(k == KD - 1),
                    )
                if use_silu:
                    s1 = spool.tile([P, 512], BF16, tag="s1", name="s1")[:, :cw]
                    nc.scalar.activation(s1, p1, AF.Silu)
                    nc.vector.tensor_mul(gt[:, h, c0 : c0 + cw], s1, p3)
                else:  # silu(a) = a * sigmoid(a); CoreSim has no Silu table
                    s1 = spool.tile([P, 512], F32, tag="s1f", name="s1f")[:, :cw]
                    s2 = spool.tile([P, 512], F32, tag="s2f", name="s2f")[:, :cw]
                    nc.scalar.activation(s1, p1, AF.Sigmoid)
                    nc.vector.tensor_mul(s2, p1, p3)
                    nc.vector.tensor_mul(gt[:, h, c0 : c0 + cw], s2, s1)

    # ---- phase 2: outT = w2.T @ gT, w2 SBUF-resident (prefetched above).
    # Stationary is a [h,128] w2 d-tile, moving is gt with tokens on the
    # free axis, so the stream is exactly n_rows columns per (d-tile, kh)
    # and the output lands d-major [D, n] (host transposes) ----
    chunks2 = []
    off = 0
    for cw in _chunk_widths(n_rows, small_first=False):
        chunks2.append((off, cw))
        off += cw
    for dt in range(D // P):
        for c0, cw in chunks2:
            po = ppo.tile([P, 512], F32, tag="po", name="po")[:, :cw]
            for kh in range(KH):
                nc.tensor.matmul(
                    po,
                    w2t[:, kh, dt * P : (dt + 1) * P],
                    gt[:, kh, c0 : c0 + cw],
                    start=(kh == 0),
                    stop=(kh == KH - 1),
                )
            ot = opool.tile([P, 512], F32, tag="ot", name="ot")[:, :cw]
            nc.vector.tensor_copy(ot, po)
            nc.sync.dma_start(
                out_ap[dt * P : (dt + 1) * P, c0 : c0 + cw],
                ot,
            )


def build_moe_program(D, H, C, S, use_silu=True):
    """SPMD program: routed expert over C capacity rows + shared expert over
    S token-slice rows. Same NEFF on all 8 cores, per-core input data."""
    nc = bacc.Bacc(
        "TRN2", target_bir_lowering=False, debug=False, num_devices=N_CORES
    )
    KD = D // P

    def din(name, shape, dt=BF16):
        return nc.dram_tensor(name, shape, dt, kind="ExternalInput").ap()

    def dout(name, shape):
        return nc.dram_tensor(name, shape, F32, kind="ExternalOutput").ap()

    xeT = din("xeT", [P, KD * C])
    xsT = din("xsT", [P, KD * S])
    w1 = din("w1", [P, KD * H])
    w3 = din("w3", [P, KD * H])
    w2 = din("w2", [P, H * D // P])
    sw1 = din("sw1", [P, KD * H])
    sw3 = din("sw3", [P, KD * H])
    sw2 = din("sw2", [P, H * D // P])
    ye = dout("ye", [D, C])
    se = dout("se", [D, S])

    with tile.TileContext(nc) as tc:
        from contextlib import ExitStack

        with ExitStack() as ctx:
            xepool = ctx.enter_context(tc.tile_pool(name="xeT", bufs=1))
            xspool = ctx.enter_context(tc.tile_pool(name="xsT", bufs=1))
            # gT and w2res are bufs=1: the shared block reuses the routed
            # block's buffer (its writes serialize after the routed phase-2
            # reads, which matches the PE's program order anyway)
            pools = (
                ctx.enter_context(tc.tile_pool(name="w2res", bufs=1)),
                ctx.enter_context(tc.tile_pool(name="wstream", bufs=4)),
                ctx.enter_context(tc.tile_pool(name="gT", bufs=1)),
                ctx.enter_context(tc.tile_pool(name="stemp", bufs=2)),
                ctx.enter_context(tc.tile_pool(name="otile", bufs=3)),
                ctx.enter_context(tc.tile_pool(name="ps1", bufs=2, space="PSUM")),
                ctx.enter_context(tc.tile_pool(name="ps3", bufs=2, space="PSUM")),
                ctx.enter_context(tc.tile_pool(name="pso", bufs=2, space="PSUM")),
            )
            # activation tiles for both blocks stay SBUF-resident; their DMAs
            # are emitted right after the first w1/w3 tile DMAs (see
            # post_w_loads) so the DMA ring serves the PE's critical path
            # first: w-hb0, then x chunk-by-chunk (narrowest chunk first)
            xet = xepool.tile([P, KD, C], BF16, tag="xeT", name="xet")
            xesrc = xeT.rearrange("p (k c) -> p k c", k=KD)
            xst = xspool.tile([P, KD, S], BF16, tag="xsT", name="xst")
            xssrc = xsT.rearrange("p (k c) -> p k c", k=KD)

            def load_acts(chunks, stage):
                if stage == 0:
                    c0, cw = chunks[0]
                    for k0 in range(0, KD, 2):
                        nc.sync.dma_start(
                            xet[:, k0 : k0 + 2, c0 : c0 + cw],
                            xesrc[:, k0 : k0 + 2, c0 : c0 + cw],
                        )
                    return
                for c0, cw in chunks[1:]:
                    for k0 in range(0, KD, 2):
                        nc.sync.dma_start(
                            xet[:, k0 : k0 + 2, c0 : c0 + cw],
                            xesrc[:, k0 : k0 + 2, c0 : c0 + cw],
                        )
                for k0 in range(0, KD, 4):
                    nc.sync.dma_start(
                        xst[:, k0 : k0 + 4, :], xssrc[:, k0 : k0 + 4, :]
                    )

            _swiglu_block(
                tc, pools, xet, C, w1, w3, w2, ye, use_silu, load_acts
            )
            _swiglu_block(tc, pools, xst, S, sw1, sw3, sw2, se, use_silu)

    nc.compile()
    return nc


_PROGRAM_CACHE = {}
LAST_RESULTS = None  # BassKernelResults of the most recent device run (for test.py)


def _get_program(D, H, C, S):
    key = (D, H, C, S)
    if key not in _PROGRAM_CACHE:
        _PROGRAM_CACHE[key] = build_moe_program(D, H, C, S)
    return _PROGRAM_CACHE[key]


def _pack_xT(xmat):
    """[n, D] row-major tokens -> [P, KD*n] partition-major k-major layout:
    element [p, k*n + c] = xmat[c, k*P + p]."""
    n, Dx = xmat.shape
    KD = Dx // P
    return np.ascontiguousarray(
        xmat.reshape(n, KD, P).transpose(2, 1, 0).reshape(P, KD * n)
    ).astype(NP_BF16)


def _pack_w13(w):
    """[D, H] -> [P, (D//P)*H] h-block-major bf16: each h-block's weights are
    one contiguous run per partition."""
    Dw, Hw = w.shape
    KD = Dw // P
    nhb = Hw // (H_BLOCK * P)
    return np.ascontiguousarray(
        w.reshape(KD, P, nhb, H_BLOCK * P)
        .transpose(1, 2, 0, 3)
        .reshape(P, KD * Hw)
    ).astype(NP_BF16)


def _pack_w2(w):
    """[H, D] -> [P, H*D//P] dn-major bf16: each 512-wide D-column half is one
    contiguous run per partition."""
    Hw, Dw = w.shape
    KH = Hw // P
    ND = Dw // 512
    return np.ascontiguousarray(
        w.reshape(KH, P, ND, 512).transpose(1, 2, 0, 3).reshape(P, Hw * Dw // P)
    ).astype(NP_BF16)


def _route(xf, w_router):
    """Top-2 routing identical (up to fp rounding) to the jax reference."""
    logits = xf @ w_router.astype(np.float32)  # [T, E]
    # softmax is monotone: top-2 of probs == top-2 of logits, stable ties
    top2 = np.argsort(-logits, axis=1, kind="stable")[:, :2]  # [T, 2]
    lv = np.take_along_axis(logits, top2, axis=1)
    ev = np.exp(lv - lv[:, 0:1])
    gates = ev / ev.sum(axis=1, keepdims=True)  # [T, 2] renormalized
    return top2, gates


def kernel(x, w_router, w1, w3, w2, sw1, sw3, sw2):
    B, SEQ, D = x.shape
    T = B * SEQ
    E, _, H = w1.shape
    assert E == N_CORES
    S = T // N_CORES

    x = np.asarray(x, dtype=np.float32)
    xf = np.ascontiguousarray(x.reshape(T, D))
    top2, gates = _route(xf, np.asarray(w_router, np.float32))

    # per-expert token lists + gate values
    flat_e = top2.ravel()  # slot 2t, 2t+1 -> token t
    flat_g = gates.ravel().astype(np.float32)
    order = np.argsort(flat_e, kind="stable")
    sorted_e = flat_e[order]
    starts = np.searchsorted(sorted_e, np.arange(E + 1))
    tok_by_e = [order[starts[e] : starts[e + 1]] >> 1 for e in range(E)]
    gate_by_e = [flat_g[order[starts[e] : starts[e + 1]]] for e in range(E)]
    counts = np.diff(starts)

    # capacity: exact max expert load (every core pays C rows of compute, so
    # don't round up); small floor keeps degenerate routings compilable
    C = max(256, int(counts.max()))

    nc = _get_program(D, H, C, S)

    w1 = np.asarray(w1, np.float32)
    w3 = np.asarray(w3, np.float32)
    w2 = np.asarray(w2, np.float32)
    sw1p = _pack_w13(np.asarray(sw1, np.float32))
    sw3p = _pack_w13(np.asarray(sw3, np.float32))
    sw2p = _pack_w2(np.asarray(sw2, np.float32))

    in_maps = []
    for e in range(E):
        n_e = int(counts[e])
        xe_pad = np.zeros((C, D), np.float32)
        xe_pad[:n_e] = xf[tok_by_e[e]]
        xeT = _pack_xT(xe_pad)
        xsT = _pack_xT(xf[e * S : (e + 1) * S])
        in_maps.append(
            {
                "xeT": xeT,
                "xsT": xsT,
                "w1": _pack_w13(w1[e]),
                "w3": _pack_w13(w3[e]),
                "w2": _pack_w2(w2[e]),
                "sw1": sw1p,
                "sw3": sw3p,
                "sw2": sw2p,
            }
        )

    global LAST_RESULTS
    LAST_RESULTS = run_bass_kernel_spmd(nc, in_maps, core_ids=list(range(N_CORES)))
    res = LAST_RESULTS.results

    # device outputs are d-major [D, n]; gates are applied here (they can't
    # broadcast along the matmul free axis on-device)
    out = np.empty((T, D), np.float32)
    for c in range(N_CORES):
        out[c * S : (c + 1) * S] = res[c]["se"].T
    for e in range(E):
        n_e = int(counts[e])
        if n_e:
            out[tok_by_e[e]] += res[e]["ye"][:, :n_e].T * gate_by_e[e][:, None]
    return out.reshape(B, SEQ, D)


# revision 39
# speedup vs baseline: 1.0270x; 1.0054x over previous
"""MoE feed-forward (top-2 routing + shared expert) on 8 Trainium2 cores.

Strategy (expert parallel):
  - Host computes the router (tiny [T,D]@[D,E] matmul), top-2 expert ids and
    renormalized gates, then dispatches each expert's tokens (transposed,
    capacity-padded) to the core that owns that expert's weights.
  - Core e computes  ye = (silu(xe@w1_e) * (xe@w3_e)) @ w2_e, row-scaled by the
    gate, plus a 1/8 token-slice of the always-active shared expert.
  - Host scatter-adds routed outputs into the shared-expert output.

All matmul operands are bf16 (fp32 PSUM accumulation), which runs the PE at
full rate AND halves HBM traffic vs f32. That lets every weight be fetched
from HBM exactly once: phase 1 iterates h-blocks in the OUTER loop (streaming
each w1/w3 tile once, all token chunks inner) while the per-token activations
and the silu(h1)*h3 gate tensor stay SBUF-resident for the whole block.
Per-core HBM traffic is ~27MB (vs ~98MB for an f32 per-chunk streaming
variant), well under the matmul time, so the kernel is tensor-engine-bound:
PE issue rate is width/2.4GHz per matmul with the ~97ns bf16 weight load
hidden (all chunk widths kept >=256), zero PE gaps >200ns after the ~12us
DMA ramp-in. Capacity is the EXACT max expert load (no 128-rounding), and
tokens ride the matmul FREE axis in BOTH phases (phase 2 keeps w2 d-tiles
stationary and streams gt, emitting d-major [D, n] outputs that the host
transposes and gate-scales), so compute scales exactly with token count --
no padding to 128-row PSUM tiles anywhere.

Accuracy: bf16 quantization of weights+activations lands at ~4e-3
absmax-relative (budget 2e-2). fp8 e4m3 was measured at 3.6-6.9e-2 for
every subset of the matmuls and is excluded.
"""

import numpy as np
import ml_dtypes

import concourse.bass as bass
import concourse.mybir as mybir
import concourse.tile as tile
from concourse import bacc
from concourse.bass_utils import run_bass_kernel_spmd

P = 128
N_CORES = 8
F32 = mybir.dt.float32
BF16 = mybir.dt.bfloat16
AF = mybir.ActivationFunctionType
NP_BF16 = ml_dtypes.bfloat16

# h-tiles of w1/w3 fetched per DMA (bigger transfers, fewer descriptors)
H_BLOCK = 2


def _chunk_widths(n, small_first=True):
    """Split n into the minimal number of chunks of width <=512 (PSUM bank),
    each >=~250 so the bf16 weight load (~97ns) hides under the column
    stream. With small_first, the first chunk is kept at 256 when possible:
    it is on the DMA critical path at kernel start."""
    m = -(-n // 512)
    if small_first and m >= 2 and n - 256 <= 512 * (m - 1):
        rest, k = n - 256, m - 1
        return [256] + [rest // k + (1 if i < rest % k else 0) for i in range(k)]
    return [n // m + (1 if i < n % m else 0) for i in range(m)]


def _swiglu_block(
    tc,
    pools,
    xt,
    n_rows,
    w1_ap,
    w3_ap,
    w2_ap,
    out_ap,
    use_silu=True,
    post_w_loads=None,
):
    """Emit one SwiGLU yT = ((silu(x@w1) * (x@w3)) @ w2).T over n_rows tokens.

    xt: SBUF tile [P, KD, n_rows] bf16 (DMA'd by caller).
    out_ap: [D, n_rows] f32 dram, d-major (host transposes; tokens live on
    the matmul free axis in both phases so compute scales exactly with
    n_rows, with no padding to a 128-row PSUM tile). Per-token gates are
    applied by the host.
    """
    nc = tc.nc
    D = out_ap.shape[0]
    KD = xt.shape[1]
    H = (w1_ap.shape[1] * P) // D
    KH = H // P
    ND = D // 512  # output free-dim tiles
    NHB = KH // H_BLOCK

    w2pool, wpool, gpool, spool, opool, pp1, pp3, ppo = pools

    hbsz = KD * H_BLOCK * P  # packed cols per h-block

    def _wsrc(ap, hb):
        return ap[:, hb * hbsz : (hb + 1) * hbsz].rearrange(
            "p (k m) -> p k m", k=KD
        )

    chunks = []
    off = 0
    for cw in _chunk_widths(n_rows):
        chunks.append((off, cw))
        off += cw

    # ---- phase 1: gT[h, c] = silu(h1T) * h3T, h-block outer so each
    # w1/w3 tile is fetched from HBM exactly once ----
    gt = gpool.tile([P, KH, n_rows], BF16, tag="gT", name="gt")
    w2t = w2pool.tile([P, KH, D], BF16, tag="w2res", name="w2t")
    for hb in range(NHB):
        w1t = wpool.tile([P, KD, H_BLOCK * P], BF16, tag="w1t", name="w1t")
        nc.sync.dma_start(w1t[:], _wsrc(w1_ap, hb))
        if hb == 0 and post_w_loads is not None:
            post_w_loads(chunks, 0)  # first x chunk: first-matmul critical path
        w3t = wpool.tile([P, KD, H_BLOCK * P], BF16, tag="w3t", name="w3t")
        nc.sync.dma_start(w3t[:], _wsrc(w3_ap, hb))
        if hb == 0 and post_w_loads is not None:
            post_w_loads(chunks, 1)  # remaining activations
        # prefetch w2 halves during the last two h-blocks so phase 2
        # starts without a DMA bubble
        for dn in range(ND):
            if hb == NHB - ND + dn:
                nc.sync.dma_start(
                    w2t[:, :, dn * 512 : (dn + 1) * 512],
                    w2_ap[:, dn * KH * 512 : (dn + 1) * KH * 512].rearrange(
                        "p (k m) -> p k m", k=KH
                    ),
                )
        for c0, cw in chunks:
            for hi in range(H_BLOCK):
                h = hb * H_BLOCK + hi
                p1 = pp1.tile([P, 512], F32, tag="p1", name="p1")[:, :cw]
                p3 = pp3.tile([P, 512], F32, tag="p3", name="p3")[:, :cw]
                for k in range(KD):
                    nc.tensor.matmul(
                        p1,
                        w1t[:, k, hi * P : (hi + 1) * P],
                        xt[:, k, c0 : c0 + cw],
                        start=(k == 0),
                        stop=(k == KD - 1),
                    )
                for k in range(KD):
                    nc.tensor.matmul(
                        p3,
                        w3t[:, k, hi * P : (hi + 1) * P],
                        xt[:, k, c0 : c0 + cw],
                        start=(k == 0),
                        stop=(k == KD - 1),
                    )
                if use_silu:
                    s1 = spool.tile([P, 512], BF16, tag="s1", name="s1")[:, :cw]
                    nc.scalar.activation(s1, p1, AF.Silu)
                    nc.vector.tensor_mul(gt[:, h, c0 : c0 + cw], s1, p3)
                else:  # silu(a) = a * sigmoid(a); CoreSim has no Silu table
                    s1 = spool.tile([P, 512], F32, tag="s1f", name="s1f")[:, :cw]
                    s2 = spool.tile([P, 512], F32, tag="s2f", name="s2f")[:, :cw]
                    nc.scalar.activation(s1, p1, AF.Sigmoid)
                    nc.vector.tensor_mul(s2, p1, p3)
                    nc.vector.tensor_mul(gt[:, h, c0 : c0 + cw], s2, s1)

    # ---- phase 2: outT = w2.T @ gT, w2 SBUF-resident (prefetched above).
    # Stationary is a [h,128] w2 d-tile, moving is gt with tokens on the
    # free axis, so the stream is exactly n_rows columns per (d-tile, kh)
    # and the output lands d-major [D, n] (host transposes) ----
    chunks2 = []
    off = 0
    for cw in _chunk_widths(n_rows, small_first=False):
        chunks2.append((off, cw))
        off += cw
    for dt in range(D // P):
        for c0, cw in chunks2:
            po = ppo.tile([P, 512], F32, tag="po", name="po")[:, :cw]
            for kh in range(KH):
                nc.tensor.matmul(
                    po,
                    w2t[:, kh, dt * P : (dt + 1) * P],
                    gt[:, kh, c0 : c0 + cw],
                    start=(kh == 0),
                    stop=(kh == KH - 1),
                )
            ot = opool.tile([P, 512], BF16, tag="ot", name="ot")[:, :cw]
            nc.vector.tensor_copy(ot, po)
            nc.sync.dma_start(
                out_ap[dt * P : (dt + 1) * P, c0 : c0 + cw],
                ot,
            )


def build_moe_program(D, H, C, S, use_silu=True):
    """SPMD program: routed expert over C capacity rows + shared expert over
    S token-slice rows. Same NEFF on all 8 cores, per-core input data."""
    nc = bacc.Bacc(
        "TRN2", target_bir_lowering=False, debug=False, num_devices=N_CORES
    )
    KD = D // P

    def din(name, shape, dt=BF16):
        return nc.dram_tensor(name, shape, dt, kind="ExternalInput").ap()

    def dout(name, shape):
        return nc.dram_tensor(name, shape, BF16, kind="ExternalOutput").ap()

    xeT = din("xeT", [P, KD * C])
    xsT = din("xsT", [P, KD * S])
    w1 = din("w1", [P, KD * H])
    w3 = din("w3", [P, KD * H])
    w2 = din("w2", [P, H * D // P])
    sw1 = din("sw1", [P, KD * H])
    sw3 = din("sw3", [P, KD * H])
    sw2 = din("sw2", [P, H * D // P])
    ye = dout("ye", [D, C])
    se = dout("se", [D, S])

    with tile.TileContext(nc) as tc:
        from contextlib import ExitStack

        with ExitStack() as ctx:
            xepool = ctx.enter_context(tc.tile_pool(name="xeT", bufs=1))
            xspool = ctx.enter_context(tc.tile_pool(name="xsT", bufs=1))
            wmpool = ctx.enter_context(tc.tile_pool(name="warm", bufs=1))
            # gT and w2res are bufs=1: the shared block reuses the routed
            # block's buffer (its writes serialize after the routed phase-2
            # reads, which matches the PE's program order anyway)
            pools = (
                ctx.enter_context(tc.tile_pool(name="w2res", bufs=1)),
                ctx.enter_context(tc.tile_pool(name="wstream", bufs=4)),
                ctx.enter_context(tc.tile_pool(name="gT", bufs=1)),
                ctx.enter_context(tc.tile_pool(name="stemp", bufs=2)),
                ctx.enter_context(tc.tile_pool(name="otile", bufs=3)),
                ctx.enter_context(tc.tile_pool(name="ps1", bufs=2, space="PSUM")),
                ctx.enter_context(tc.tile_pool(name="ps3", bufs=2, space="PSUM")),
                ctx.enter_context(tc.tile_pool(name="pso", bufs=2, space="PSUM")),
            )
            # activation tiles for both blocks stay SBUF-resident; their DMAs
            # are emitted right after the first w1/w3 tile DMAs (see
            # post_w_loads) so the DMA ring serves the PE's critical path
            # first: w-hb0, then x chunk-by-chunk (narrowest chunk first)
            xet = xepool.tile([P, KD, C], BF16, tag="xeT", name="xet")
            xesrc = xeT.rearrange("p (k c) -> p k c", k=KD)
            xst = xspool.tile([P, KD, S], BF16, tag="xsT", name="xst")
            xssrc = xsT.rearrange("p (k c) -> p k c", k=KD)

            # a short dummy-matmul burst (touches no DMA queue) so the PE
            # p-state ramps during the input-DMA wait; it ends well before
            # the first weight tile lands
            warm = wmpool.tile([P, 512], BF16, tag="wm", name="warm")
            nc.vector.memset(warm[:], 0)
            pwd = pools[-1].tile([P, 512], F32, tag="po", name="pwd")
            for i in range(8):
                nc.tensor.matmul(
                    pwd, warm[:, :P], warm[:], start=(i == 0), stop=(i == 7)
                )

            def load_acts(chunks, stage):
                if stage == 0:
                    c0, cw = chunks[0]
                    for k0 in range(0, KD, 2):
                        nc.sync.dma_start(
                            xet[:, k0 : k0 + 2, c0 : c0 + cw],
                            xesrc[:, k0 : k0 + 2, c0 : c0 + cw],
                        )
                    return
                for c0, cw in chunks[1:]:
                    for k0 in range(0, KD, 2):
                        nc.sync.dma_start(
                            xet[:, k0 : k0 + 2, c0 : c0 + cw],
                            xesrc[:, k0 : k0 + 2, c0 : c0 + cw],
                        )
                for k0 in range(0, KD, 4):
                    nc.sync.dma_start(
                        xst[:, k0 : k0 + 4, :], xssrc[:, k0 : k0 + 4, :]
                    )

            _swiglu_block(
                tc, pools, xet, C, w1, w3, w2, ye, use_silu, load_acts
            )
            _swiglu_block(tc, pools, xst, S, sw1, sw3, sw2, se, use_silu)

    nc.compile()
    return nc


_PROGRAM_CACHE = {}
LAST_RESULTS = None  # BassKernelResults of the most recent device run (for test.py)


def _get_program(D, H, C, S):
    key = (D, H, C, S)
    if key not in _PROGRAM_CACHE:
        _PROGRAM_CACHE[key] = build_moe_program(D, H, C, S)
    return _PROGRAM_CACHE[key]


def _pack_xT(xmat):
    """[n, D] row-major tokens -> [P, KD*n] partition-major k-major layout:
    element [p, k*n + c] = xmat[c, k*P + p]."""
    n, Dx = xmat.shape
    KD = Dx // P
    return np.ascontiguousarray(
        xmat.reshape(n, KD, P).transpose(2, 1, 0).reshape(P, KD * n)
    ).astype(NP_BF16)


def _pack_w13(w):
    """[D, H] -> [P, (D//P)*H] h-block-major bf16: each h-block's weights are
    one contiguous run per partition."""
    Dw, Hw = w.shape
    KD = Dw // P
    nhb = Hw // (H_BLOCK * P)
    return np.ascontiguousarray(
        w.reshape(KD, P, nhb, H_BLOCK * P)
        .transpose(1, 2, 0, 3)
        .reshape(P, KD * Hw)
    ).astype(NP_BF16)


def _pack_w2(w):
    """[H, D] -> [P, H*D//P] dn-major bf16: each 512-wide D-column half is one
    contiguous run per partition."""
    Hw, Dw = w.shape
    KH = Hw // P
    ND = Dw // 512
    return np.ascontiguousarray(
        w.reshape(KH, P, ND, 512).transpose(1, 2, 0, 3).reshape(P, Hw * Dw // P)
    ).astype(NP_BF16)


def _route(xf, w_router):
    """Top-2 routing identical (up to fp rounding) to the jax reference."""
    logits = xf @ w_router.astype(np.float32)  # [T, E]
    # softmax is monotone: top-2 of probs == top-2 of logits, stable ties
    top2 = np.argsort(-logits, axis=1, kind="stable")[:, :2]  # [T, 2]
    lv = np.take_along_axis(logits, top2, axis=1)
    ev = np.exp(lv - lv[:, 0:1])
    gates = ev / ev.sum(axis=1, keepdims=True)  # [T, 2] renormalized
    return top2, gates


def kernel(x, w_router, w1, w3, w2, sw1, sw3, sw2):
    B, SEQ, D = x.shape
    T = B * SEQ
    E, _, H = w1.shape
    assert E == N_CORES
    S = T // N_CORES

    x = np.asarray(x, dtype=np.float32)
    xf = np.ascontiguousarray(x.reshape(T, D))
    top2, gates = _route(xf, np.asarray(w_router, np.float32))

    # per-expert token lists + gate values
    flat_e = top2.ravel()  # slot 2t, 2t+1 -> token t
    flat_g = gates.ravel().astype(np.float32)
    order = np.argsort(flat_e, kind="stable")
    sorted_e = flat_e[order]
    starts = np.searchsorted(sorted_e, np.arange(E + 1))
    tok_by_e = [order[starts[e] : starts[e + 1]] >> 1 for e in range(E)]
    gate_by_e = [flat_g[order[starts[e] : starts[e + 1]]] for e in range(E)]
    counts = np.diff(starts)

    # capacity: exact max expert load (every core pays C rows of compute, so
    # don't round up); small floor keeps degenerate routings compilable
    C = max(256, int(counts.max()))

    nc = _get_program(D, H, C, S)

    w1 = np.asarray(w1, np.float32)
    w3 = np.asarray(w3, np.float32)
    w2 = np.asarray(w2, np.float32)
    sw1p = _pack_w13(np.asarray(sw1, np.float32))
    sw3p = _pack_w13(np.asarray(sw3, np.float32))
    sw2p = _pack_w2(np.asarray(sw2, np.float32))

    in_maps = []
    for e in range(E):
        n_e = int(counts[e])
        xe_pad = np.zeros((C, D), np.float32)
        xe_pad[:n_e] = xf[tok_by_e[e]]
        xeT = _pack_xT(xe_pad)
        xsT = _pack_xT(xf[e * S : (e + 1) * S])
        in_maps.append(
            {
                "xeT": xeT,
                "xsT": xsT,
                "w1": _pack_w13(w1[e]),
                "w3": _pack_w13(w3[e]),
                "w2": _pack_w2(w2[e]),
                "sw1": sw1p,
                "sw3": sw3p,
                "sw2": sw2p,
            }
        )

    global LAST_RESULTS
    LAST_RESULTS = run_bass_kernel_spmd(nc, in_maps, core_ids=list(range(N_CORES)))
    res = LAST_RESULTS.results

    # device outputs are d-major [D, n]; gates are applied here (they can't
    # broadcast along the matmul free axis on-device)
    out = np.empty((T, D), np.float32)
    for c in range(N_CORES):
        out[c * S : (c + 1) * S] = np.asarray(res[c]["se"], np.float32).T
    for e in range(E):
        n_e = int(counts[e])
        if n_e:
            ye = np.asarray(res[e]["ye"][:, :n_e], np.float32).T
            out[tok_by_e[e]] += ye * gate_by_e[e][:, None]
    return out.reshape(B, SEQ, D)
